# revision 1
# baseline (speedup 1.0000x reference)
"""Bass/Trainium2 kernel for nn_MultiHeadAttention_85615878078828.

Full (unsharded) inputs in, full output out. Sharding: 8 cores =
2 batches x 4 head-groups (tensor-parallel on heads + data-parallel on
batch). Each core runs QKV projection + causal attention for its 4
heads + a partial output projection; the host sums the 4 partial
projections per batch element (the "all-reduce" of the TP out-proj,
done during unshard) and adds b_out.

Everything on-device is computed in transposed layout ([dim, seq]) so
no on-device transposes of activations are needed:
  - QKV_T  = W-stationary matmuls against x^T (host pre-transposes x)
  - scores = S_T[k, q] directly (2 heads packed on the PE array via
    row tiling, contract dim = 64 each)
  - causal mask folded into PSUM with an identity-matmul init
  - exp on ScalarE with fused 1/sqrt(D) scale, P_T stored bf16
  - P@V with ones-augmented V (M=65) -> softmax denominators fall out
    of the same accumulation (row 64)
  - normalization: reciprocal + gpsimd partition-broadcast + multiply
  - out-proj in transposed layout; host transposes back.
"""

import os
import sys
from contextlib import ExitStack

import numpy as np

sys.path.insert(0, "/opt/trn_rl_repo")

import ml_dtypes

BF16NP = ml_dtypes.bfloat16
F8NP = ml_dtypes.float8_e4m3

import concourse.bass as bass
import concourse.tile as tile
from concourse import bacc, mybir
from concourse.bass_utils import run_bass_kernel_spmd

# Problem shapes (hardcoded per contract)
B, S, H, N = 2, 2048, 1024, 16
D = H // N  # 64
P = 128
SCALE = float(D) ** -0.5  # 0.125
NEG = -1.0e30

F32 = mybir.dt.float32
F32R = mybir.dt.float32r
BF16 = mybir.dt.bfloat16
F16 = mybir.dt.float16
FP8 = mybir.dt.float8e4
DR = mybir.MatmulPerfMode.DoubleRow
DEBUG = False


def _emit(nc, tc, ctx):
    add = mybir.AluOpType.add
    mult = mybir.AluOpType.mult
    Exp = mybir.ActivationFunctionType.Exp

    # ---- DRAM I/O ----
    xT = nc.dram_tensor("xT", [H, S], BF16, kind="ExternalInput").ap()
    wqkT = nc.dram_tensor("wqkT", [H, 512], BF16, kind="ExternalInput").ap()
    wvT = nc.dram_tensor("wvT", [H, 256], BF16, kind="ExternalInput").ap()
    bqkv = nc.dram_tensor("bqkv", [768], F32, kind="ExternalInput").ap()
    woT = nc.dram_tensor("woT", [256, H], BF16, kind="ExternalInput").ap()
    maskd = nc.dram_tensor("maskd", [P, P], BF16, kind="ExternalInput").ap()
    identd = nc.dram_tensor("identd", [P, P], BF16, kind="ExternalInput").ap()
    outT = nc.dram_tensor("outT", [H, S], F16, kind="ExternalOutput").ap()
    vdram = [nc.dram_tensor(f"vdram{i}", [P, S], BF16).ap() for i in range(2)]

    # ---- pools (single scope; SBUF is sized to fit everything) ----
    const = ctx.enter_context(tc.tile_pool(name="const", bufs=1))
    qkp = ctx.enter_context(tc.tile_pool(name="qk", bufs=1))
    vaugp = ctx.enter_context(tc.tile_pool(name="vaug", bufs=1))
    attnp = ctx.enter_context(tc.tile_pool(name="attn", bufs=1))
    smallp = ctx.enter_context(tc.tile_pool(name="small", bufs=3))
    oevacp = ctx.enter_context(tc.tile_pool(name="oevac", bufs=4))
    xp = ctx.enter_context(tc.tile_pool(name="xp", bufs=1))
    vtp = ctx.enter_context(tc.tile_pool(name="vtp", bufs=2))
    pp = ctx.enter_context(tc.tile_pool(name="pp", bufs=1))
    # PSUM: scores 2x(2 banks) + shared small-psum 4x(1 bank) = 8 banks
    scps = ctx.enter_context(tc.tile_pool(name="scps", bufs=2, space="PSUM"))
    psml = ctx.enter_context(tc.tile_pool(name="psml", bufs=4, space="PSUM"))

    # weights + small constants first so QKV compute can begin as soon as
    # the first x s-block lands; x streams per s-block after.
    b_sb = xp.tile([P, 6], F32, tag="b")
    nc.sync.dma_start(b_sb[:], bqkv.rearrange("(c p) -> p c", p=P))
    wqk_sb = xp.tile([P, 8, 512], BF16, tag="wqk")
    _wqk_r = wqkT.rearrange("(o p) c -> p o c", p=P)
    nc.sync.dma_start(wqk_sb[:], _wqk_r)
    wv_sb = xp.tile([P, 8, 256], BF16, tag="wv")
    _wv_r = wvT.rearrange("(o p) c -> p o c", p=P)
    nc.sync.dma_start(wv_sb[:], _wv_r)
    # one tile PER s-block: DMA-write -> matmul-read dependencies are
    # tracked per tile, so a single x tile would gate every QKV group
    # on the LAST x DMA (measured: first matmul sat idle 7.7us for it)
    x_t = [
        xp.tile([P, 8, 512], BF16, tag=f"x{sb}", name=f"x{sb}")
        for sb in range(4)
    ]
    x_r = xT.rearrange("(o p) s -> p o s", p=P)

    def x_load(sb):
        nc.sync.dma_start(
            x_t[sb][:], x_r[:, :, 512 * sb : 512 * (sb + 1)]
        )

    wo_sb = const.tile([P, 2, H], BF16, tag="wo")
    mask_sb = const.tile([P, P], BF16, tag="mask")
    id_sb = const.tile([P, P], BF16, tag="ident")

    def late_loads():
        nc.sync.dma_start(mask_sb[:], maskd)
        nc.sync.dma_start(id_sb[:], identd)
        nc.sync.dma_start(wo_sb[:], woT.rearrange("(c p) j -> p c j", p=P))


    qt = [qkp.tile([P, S], BF16, tag=f"qt{i}", name=f"qt{i}") for i in range(2)]
    kt = [qkp.tile([P, S], BF16, tag=f"kt{i}", name=f"kt{i}") for i in range(2)]
    vt = [vtp.tile([P, S], BF16, tag="vt", name=f"vt{i}") for i in range(2)]
    # ones-augmented V: fp8 copy (padded to stride 80 -- DoubleRow needs
    # the chunk stride to be a multiple of 16 bytes) for the bulk P@V,
    # plus a bf16 copy of key chunks 0-3 for the early query blocks
    # whose peaked softmax can't average out fp8 noise.
    vaug = [vaugp.tile([P, 16, 80], FP8, tag=f"vaug{h}", name=f"vaug{h}") for h in range(4)]
    vaug_hi = [vaugp.tile([P, 4, 65], BF16, tag=f"vaughi{h}", name=f"vaughi{h}") for h in range(4)]
    for h in range(4):
        nc.gpsimd.memset(vaug[h][:, :, 64:65], 1.0)
        nc.gpsimd.memset(vaug_hi[h][:, :, 64:65], 1.0)
    attn = [attnp.tile([P, S], BF16, tag=f"attn{i}", name=f"attn{i}") for i in range(2)]

    def qkv_group(pair, ci, sb):
        """One (chunk, s-block) accumulation of the QKV projection."""
        pc = [pair, 2 + pair, 4 + pair][ci]
        if pc < 4:
            w_ch = wqk_sb[:, :, 128 * pc : 128 * (pc + 1)]
        else:
            w_ch = wv_sb[:, :, 128 * (pc - 4) : 128 * (pc - 3)]
        dst = [qt[pair], kt[pair], vt[pair]][ci]
        ps = psml.tile([P, 512], F32, tag="psml", name="psml")
        for o in range(8):
            nc.tensor.matmul(
                ps[:],
                w_ch[:, o, :],
                x_t[sb][:, o, :],
                start=(o == 0),
                stop=(o == 7),
            )
        nc.vector.tensor_scalar(
            out=dst[:, 512 * sb : 512 * (sb + 1)],
            in0=ps[:],
            scalar1=b_sb[:, pc : pc + 1],
            scalar2=None,
            op0=add,
        )

    def v_finish(pair):
        """V_T -> V[k,d] via DRAM-bounce xbar transpose (contiguous dst),
        then engine-copy into the ones-augmented tiles."""
        nc.sync.dma_start(vdram[pair], vt[pair][:])
        for sub in range(2):
            h = 2 * pair + sub
            vkd = vtp.tile([P, 16, 64], BF16, tag="vkd", name=f"vkd{h}")
            nc.sync.dma_start_transpose(
                vkd[:], vdram[pair][64 * sub : 64 * sub + 64, :]
            )
            nc.vector.tensor_copy(vaug[h][:, :, 0:64], vkd[:])
            nc.vector.tensor_copy(vaug_hi[h][:, :, 0:64], vkd[:, 0:4, :])

    def filler_items(pair, T):
        for ci in range(3):
            for sb in range(4):
                yield ("qkv", pair, ci, sb)
        yield ("vfin", pair)
        for kc in range(4):
            yield ("esc", pair, kc, T)

    def emit_filler(it):
        if it is None:
            return
        if it[0] == "qkv":
            qkv_group(it[1], it[2], it[3])
        elif it[0] == "op":
            outproj_group(it[1], it[2], it[3])
        elif it[0] == "opdma":
            outproj_dma(it[1], it[2])
        elif it[0] == "esc":
            score_chunk(it[1], it[2], it[3])
        else:
            v_finish(it[1])

    def make_pt_tiles(pair):
        """P_T tiles for one pair: fp8 fixed-stride storage for key
        chunks 4-15 (pt), bf16 for chunks 0-3 x cols [0,1024) (pt_hi,
        full precision for the early query blocks whose peaked softmax
        can't average out fp8 noise), and per-pair fp8 scratch for
        chunks 0-3 x cols [1024,2048) (ptx) so the next pair's lead
        chunks can be computed early without aliasing this pair's."""
        pt = [
            pp.tile([P, 12, S], FP8, tag=f"pt{s}", name=f"pt{s}")
            for s in range(2)
        ]
        pt_hi = [
            pp.tile([P, 4, 1024], BF16, tag=f"pthi{s}", name=f"pthi{s}")
            for s in range(2)
        ]
        ptx = [
            pp.tile([P, 4, 1024], FP8, tag=f"ptx{pair}{s}", name=f"ptx{s}")
            for s in range(2)
        ]
        return pt, pt_hi, ptx

    def score_chunk(pair, kc, T):
        """Scores + exp + strip-zeroing for one 128-key chunk."""
        pt, pt_hi, ptx = T
        j = kc // 4
        d = kc - 4 * j
        g0 = (128 * kc) // 1024
        st = {}
        for sub in range(2):
            for g in range(g0, 2):
                st[sub, g] = scps.tile([P, 1024], F32, tag="st", name="st")
        # causal-mask init of the diagonal 128x128 block
        # (start=True clears the whole containing bank)
        lc = 128 * kc - 1024 * g0
        for sub in range(2):
            nc.tensor.matmul(
                st[sub, g0][:, lc : lc + 128],
                id_sb,
                mask_sb,
                start=True,
                stop=False,
            )
        # scores S_T[k, q], heads interleaved (PE row packing).
        # Diagonal block split at the 128-col boundary: first 128
        # cols accumulate onto the mask init, the rest of the bank
        # overwrites (has_written clear).
        for jb in range(j, 4):
            segs = []
            if jb == j:
                segs.append((128 * kc, 128, False, d == 3))
                if d < 3:
                    segs.append(
                        (128 * kc + 128, 512 * (j + 1) - 128 * kc - 128,
                         False, True)
                    )
            else:
                segs.append((512 * jb, 512, True, True))
            for n0, ln, sflag, eflag in segs:
                g = n0 // 1024
                l0 = n0 - 1024 * g
                for sub in range(2):
                    o0 = 64 * sub
                    nc.tensor.matmul(
                        st[sub, g][:, l0 : l0 + ln],
                        kt[pair][o0 : o0 + 64, 128 * kc : 128 * kc + 128],
                        qt[pair][o0 : o0 + 64, n0 : n0 + ln],
                        start=sflag,
                        stop=eflag,
                    )
        # exp (scaled) PSUM -> P_T
        for sub in range(2):
            for g in range(g0, 2):
                l0 = max(0, 128 * kc - 1024 * g)
                q0 = 1024 * g + l0
                gl = 1024 - l0
                if kc < 4:
                    dst = (
                        pt_hi[sub][:, kc, q0 : q0 + gl]
                        if g == 0
                        else ptx[sub][:, kc, q0 - 1024 : q0 - 1024 + gl]
                    )
                else:
                    dst = pt[sub][:, kc - 4, q0 : q0 + gl]
                nc.scalar.activation(
                    dst,
                    st[sub, g][:, l0 : l0 + gl],
                    Exp,
                    scale=SCALE,
                )
        # zero the below-diagonal strip [512j, 128kc)
        if d > 0:
            for sub in range(2):
                if kc < 4:
                    nc.gpsimd.memset(pt_hi[sub][:, kc, 0 : 128 * kc], 0.0)
                else:
                    nc.gpsimd.memset(
                        pt[sub][:, kc - 4, 512 * j : 128 * kc], 0.0
                    )

    def attention(pair, filler, T, per_j=None, lead=0, fills=None):
        """Attention for one head pair; pulls filler work between score
        chunks to keep the PE warm while ScalarE works through the exp
        backlog. lead: number of chunks already emitted earlier (as
        filler inside the previous pair's attention / right after it)."""
        for j in range(4):
            for kc in range(4 * j, 4 * j + 4):
                if kc >= lead:
                    score_chunk(pair, kc, T)
                for _ in range(fills[kc] if fills else 1):
                    emit_filler(next(filler, None))
            pv_block(pair, j, T)
            if per_j is not None:
                per_j(j)

    def pv_block(pair, j, T):
        pt, pt_hi, ptx = T
        if True:
            # P@V for query block j (plus denominator row 64). Key
            # chunks 0-3 of the first two query blocks run in bf16 from
            # pt_hi; the rest contract fp8 chunk-pairs via DoubleRow.
            for sub in range(2):
                h = 2 * pair + sub
                pv = psml.tile([P, 512], F32, tag="psml", name="pv")
                if j < 2:
                    for kc in range(4):
                        nc.tensor.matmul(
                            pv[0:65, :],
                            vaug_hi[h][:, kc, :],
                            pt_hi[sub][:, kc, 512 * j : 512 * (j + 1)],
                            start=(kc == 0),
                            stop=(j == 0 and kc == 3),
                        )
                for kp in range(2 * j + 2):
                    if kp < 2 and j < 2:
                        continue
                    if kp < 2:
                        rhs = ptx[sub][:, 2 * kp : 2 * kp + 2,
                                       512 * (j - 2) : 512 * (j - 1)]
                    else:
                        rhs = pt[sub][:, 2 * kp - 4 : 2 * kp - 2,
                                      512 * j : 512 * (j + 1)]
                    nc.tensor.matmul(
                        pv[0:65, :],
                        vaug[h][:, 2 * kp : 2 * kp + 2, 0:65],
                        rhs,
                        start=(kp == 0 and j >= 2),
                        stop=(kp == 2 * j + 1),
                        perf_mode=DR,
                    )
                sums = smallp.tile([1, 512], F32, tag="sums", name="sums")
                nc.vector.tensor_copy(sums[:], pv[64:65, :])
                rec = smallp.tile([1, 512], F32, tag="rec", name="rec")
                nc.vector.reciprocal_approx_fast(rec[:], sums[:])
                rbc = smallp.tile([64, 512], F32, tag="rbc", name="rbc")
                nc.gpsimd.partition_broadcast(rbc[:], rec[:])
                nc.vector.tensor_tensor(
                    out=attn[pair][64 * sub : 64 * sub + 64, 512 * j : 512 * (j + 1)],
                    in0=pv[0:64, :],
                    in1=rbc[:],
                    op=mult,
                )

    o_r = outT.rearrange("(o p) s -> p o s", p=P)

    def outproj_group(jc, sb, ev):
        """One (j-chunk, s-block) of the partial output projection."""
        ps = psml.tile([P, 512], F32, tag="psml", name="ops")
        for pc2 in range(2):
            nc.tensor.matmul(
                ps[:],
                wo_sb[:, pc2, 128 * jc : 128 * (jc + 1)],
                attn[pc2][:, 512 * sb : 512 * (sb + 1)],
                start=(pc2 == 0),
                stop=(pc2 == 1),
            )
        nc.vector.tensor_copy(ev[:, jc, :], ps[:])

    def outproj_dma(sb, ev):
        nc.sync.dma_start(o_r[:, :, 512 * sb : 512 * (sb + 1)], ev[:])

    # pair 0: QKV streamed per s-block right behind its x DMA, attention
    # with pair-1 QKV + pair-1's lead score chunks as PE filler.
    # pair 1: attention with the out-projection as PE filler -- each
    # finished s-block's 8 projection groups are queued and drained
    # through the next blocks' score slots, so the in-order PE never
    # stalls behind a normalization chain.
    for sb in range(4):
        x_load(sb)
        for ci in range(3):
            qkv_group(0, ci, sb)
    late_loads()
    v_finish(0)
    T0 = make_pt_tiles(0)
    T1 = make_pt_tiles(1)
    f1 = filler_items(1, T1)
    attention(0, f1, T0, fills=[1] * 8 + [2] * 8)
    for it in f1:
        emit_filler(it)
    # pair 1's chunks 4-7 right behind pair 0's last P@V: their exps
    # overwrite pair-0 P_T regions, so they can't go earlier, but
    # emitting them here keeps PE and ScalarE primed through the
    # pair transition.
    for kc in range(4, 8):
        score_chunk(1, kc, T1)

    import collections as _collections
    import itertools as _itertools

    oq = _collections.deque()

    def push_outproj(j):
        ev = oevacp.tile([P, 8, 512], F16, tag="evb", name=f"evb{j}", bufs=2)
        oq.extend(("op", jc, j, ev) for jc in range(8))
        oq.append(("opdma", j, ev))

    attention(
        1,
        ((oq.popleft() if oq else None) for _ in _itertools.count()),
        T1,
        per_j=push_outproj,
        lead=8,
        fills=[2] * 16,
    )
    while oq:
        emit_filler(oq.popleft())


_NC_CACHE = {}


def build_nc():
    if "nc" in _NC_CACHE:
        return _NC_CACHE["nc"]
    nc = bacc.Bacc(
        "TRN2",
        target_bir_lowering=False,
        debug=False,
        num_devices=8,
    )
    with tile.TileContext(nc) as tc:
        with ExitStack() as ctx:
            _emit(nc, tc, ctx)
    nc.compile()
    _NC_CACHE["nc"] = nc
    return nc


def make_in_maps(hidden_states, w_in, b_in, w_out):
    hidden_states = np.asarray(hidden_states, dtype=np.float32)
    w_in = np.asarray(w_in, dtype=np.float32)
    b_in = np.asarray(b_in, dtype=np.float32)
    w_out = np.asarray(w_out, dtype=np.float32)

    xT = [np.ascontiguousarray(hidden_states[b].T).astype(BF16NP) for b in range(B)]
    mask = np.where(
        np.arange(P)[:, None] <= np.arange(P)[None, :], 0.0, NEG
    ).astype(BF16NP)
    ident = np.eye(P, dtype=BF16NP)

    in_maps = []
    for c in range(8):
        b, hg = divmod(c, 4)
        q0 = 256 * hg
        wq = w_in[q0 : q0 + 256]
        wk = w_in[H + q0 : H + q0 + 256]
        wv = w_in[2 * H + q0 : 2 * H + q0 + 256]
        in_maps.append(
            {
                "xT": xT[b],
                "wqkT": np.ascontiguousarray(
                    np.concatenate([wq, wk], 0).T
                ).astype(BF16NP),
                "wvT": np.ascontiguousarray(wv.T).astype(BF16NP),
                "bqkv": np.ascontiguousarray(
                    np.concatenate(
                        [b_in[q0 : q0 + 256], b_in[H + q0 : H + q0 + 256],
                         b_in[2 * H + q0 : 2 * H + q0 + 256]]
                    )
                ),
                "woT": np.ascontiguousarray(w_out[:, q0 : q0 + 256].T).astype(BF16NP),
                "maskd": mask,
                "identd": ident,
            }
        )
    return in_maps


def _ensure_ntff_hook():
    """Provide antenv.axon_hooks (NTFF profiling hook) if the container's
    antenv stub lacks it, by driving the axon .so C ABI directly. Also
    neuter the S3 artifact upload (zero-egress container)."""
    import contextlib
    import ctypes
    import types

    import concourse.bass_utils as bu

    bu.upload_artifacts = lambda tmpdir: str(tmpdir)
    try:
        from antenv.axon_hooks import get_axon_ntff_profile_hook  # noqa: F401

        return
    except ImportError:
        pass
    import antenv

    so_path = "/opt/axon/libaxon_pjrt.so"
    hook = None
    try:
        lib = ctypes.CDLL(so_path)
        if hasattr(lib, "axon_start_nrt_profile"):
            lib.axon_start_nrt_profile.argtypes = [
                ctypes.POINTER(ctypes.c_int64),
                ctypes.c_size_t,
            ]
            lib.axon_start_nrt_profile.restype = ctypes.c_int64
            lib.axon_stop_nrt_profile.argtypes = [ctypes.c_char_p]
            lib.axon_stop_nrt_profile.restype = ctypes.c_int64

            @contextlib.contextmanager
            def _hook(output_dir, device_ids):
                import jax

                jax.devices()
                if device_ids:
                    ids = (ctypes.c_int64 * len(device_ids))(*device_ids)
                    rc = lib.axon_start_nrt_profile(ids, len(device_ids))
                else:
                    rc = lib.axon_start_nrt_profile(None, 0)
                if rc != 0:
                    raise RuntimeError(f"axon_start_nrt_profile rc={rc}")
                try:
                    yield
                finally:
                    n = lib.axon_stop_nrt_profile(str(output_dir).encode())
                    print(f"ntff profile: {n} file(s) -> {output_dir}")

            hook = _hook
    except OSError:
        hook = None

    mod = types.ModuleType("antenv.axon_hooks")
    mod.get_axon_ntff_profile_hook = lambda: hook
    mod.set_axon_ntff_profile_hook = lambda h: None
    sys.modules["antenv.axon_hooks"] = mod
    antenv.axon_hooks = mod


def run_device(hidden_states, w_in, b_in, w_out, b_out, trace=False):
    """Returns (full output, BassKernelResults)."""
    if trace:
        _ensure_ntff_hook()
    nc = build_nc()
    in_maps = make_in_maps(hidden_states, w_in, b_in, w_out)
    res = run_bass_kernel_spmd(
        nc, in_maps, core_ids=list(range(8)), trace=trace
    )
    out = np.zeros((B, S, H), dtype=np.float32)
    for c in range(8):
        out[c // 4] += res.results[c]["outT"].T
    out += np.asarray(b_out, dtype=np.float32)[None, None, :]
    return out, res


def kernel(hidden_states, w_in, b_in, w_out, b_out):
    out, _ = run_device(hidden_states, w_in, b_in, w_out, b_out, trace=False)
    return out



# revision 5
# speedup vs baseline: 1.0528x; 1.0528x over previous
"""Bass/Trainium2 kernel for nn_MultiHeadAttention_85615878078828.

Full (unsharded) inputs in, full output out. Sharding: 8 cores =
2 batches x 4 head-groups (tensor-parallel on heads + data-parallel on
batch). Each core runs QKV projection + causal attention for its 4
heads + a partial output projection; the host sums the 4 partial
projections per batch element (the "all-reduce" of the TP out-proj,
done during unshard) and adds b_out.

Everything on-device is computed in transposed layout ([dim, seq]) so
no on-device transposes of activations are needed:
  - QKV_T  = W-stationary matmuls against x^T (host pre-transposes x)
  - scores = S_T[k, q] directly (2 heads packed on the PE array via
    row tiling, contract dim = 64 each)
  - causal mask folded into PSUM with an identity-matmul init
  - exp on ScalarE with fused 1/sqrt(D) scale, P_T stored bf16
  - P@V with ones-augmented V (M=65) -> softmax denominators fall out
    of the same accumulation (row 64)
  - normalization: reciprocal + gpsimd partition-broadcast + multiply
  - out-proj in transposed layout; host transposes back.
"""

import os
import sys
from contextlib import ExitStack

import numpy as np

sys.path.insert(0, "/opt/trn_rl_repo")

import ml_dtypes

BF16NP = ml_dtypes.bfloat16
F8NP = ml_dtypes.float8_e4m3

import concourse.bass as bass
import concourse.tile as tile
from concourse import bacc, mybir
from concourse.bass_utils import run_bass_kernel_spmd

# Problem shapes (hardcoded per contract)
B, S, H, N = 2, 2048, 1024, 16
D = H // N  # 64
P = 128
SCALE = float(D) ** -0.5  # 0.125
NEG = -1.0e30

F32 = mybir.dt.float32
F32R = mybir.dt.float32r
BF16 = mybir.dt.bfloat16
F16 = mybir.dt.float16
FP8 = mybir.dt.float8e4
DR = mybir.MatmulPerfMode.DoubleRow
DEBUG = False
NWARM = 14


def _emit(nc, tc, ctx):
    add = mybir.AluOpType.add
    mult = mybir.AluOpType.mult
    Exp = mybir.ActivationFunctionType.Exp

    # ---- DRAM I/O ----
    xT = nc.dram_tensor("xT", [H, S], BF16, kind="ExternalInput").ap()
    wqkT = nc.dram_tensor("wqkT", [H, 512], BF16, kind="ExternalInput").ap()
    wvT = nc.dram_tensor("wvT", [H, 256], BF16, kind="ExternalInput").ap()
    bqkv = nc.dram_tensor("bqkv", [768], F32, kind="ExternalInput").ap()
    woT = nc.dram_tensor("woT", [256, H], BF16, kind="ExternalInput").ap()
    maskd = nc.dram_tensor("maskd", [P, P], BF16, kind="ExternalInput").ap()
    identd = nc.dram_tensor("identd", [P, P], BF16, kind="ExternalInput").ap()
    outT = nc.dram_tensor("outT", [H, S], F16, kind="ExternalOutput").ap()
    vdram = [nc.dram_tensor(f"vdram{i}", [P, S], BF16).ap() for i in range(2)]

    # ---- pools (single scope; SBUF is sized to fit everything) ----
    const = ctx.enter_context(tc.tile_pool(name="const", bufs=1))
    qkp = ctx.enter_context(tc.tile_pool(name="qk", bufs=1))
    vaugp = ctx.enter_context(tc.tile_pool(name="vaug", bufs=1))
    attnp = ctx.enter_context(tc.tile_pool(name="attn", bufs=1))
    smallp = ctx.enter_context(tc.tile_pool(name="small", bufs=3))
    oevacp = ctx.enter_context(tc.tile_pool(name="oevac", bufs=4))
    xp = ctx.enter_context(tc.tile_pool(name="xp", bufs=1))
    vtp = ctx.enter_context(tc.tile_pool(name="vtp", bufs=2))
    pp = ctx.enter_context(tc.tile_pool(name="pp", bufs=1))
    # PSUM: scores 2x(2 banks) + shared small-psum 4x(1 bank) = 8 banks
    scps = ctx.enter_context(tc.tile_pool(name="scps", bufs=2, space="PSUM"))
    psml = ctx.enter_context(tc.tile_pool(name="psml", bufs=4, space="PSUM"))

    # DMA order is tuned so the first QKV group's inputs (b, q-weight
    # chunk for pair 0, x s-block 0) land first; everything else
    # streams behind at full HBM rate. Weight chunks live in separate
    # tiles because Tile tracks DMA->matmul deps per tile.
    b_sb = xp.tile([P, 6], F32, tag="b")
    nc.sync.dma_start(b_sb[:], bqkv.rearrange("(c p) -> p c", p=P))
    _wqk_r = wqkT.rearrange("(o p) c -> p o c", p=P)
    _wv_r = wvT.rearrange("(o p) c -> p o c", p=P)
    wqk_c = [
        xp.tile([P, 8, 128], BF16, tag=f"wqk{pc}", name=f"wqk{pc}")
        for pc in range(4)
    ]
    wv_c = [
        xp.tile([P, 8, 128], BF16, tag=f"wv{pc}", name=f"wv{pc}")
        for pc in range(2)
    ]

    def w_load(pc):
        if pc < 4:
            nc.sync.dma_start(
                wqk_c[pc][:], _wqk_r[:, :, 128 * pc : 128 * (pc + 1)]
            )
        else:
            nc.sync.dma_start(
                wv_c[pc - 4][:], _wv_r[:, :, 128 * (pc - 4) : 128 * (pc - 3)]
            )

    # one tile PER s-block: DMA-write -> matmul-read dependencies are
    # tracked per tile, so a single x tile would gate every QKV group
    # on the LAST x DMA (measured: first matmul sat idle 7.7us for it)
    x_t = [
        xp.tile([P, 8, 512], BF16, tag=f"x{sb}", name=f"x{sb}")
        for sb in range(4)
    ]
    x_r = xT.rearrange("(o p) s -> p o s", p=P)

    def x_load(sb):
        nc.sync.dma_start(
            x_t[sb][:], x_r[:, :, 512 * sb : 512 * (sb + 1)]
        )

    wo_sb = const.tile([P, 2, H], BF16, tag="wo")
    mask_sb = const.tile([P, P], BF16, tag="mask")
    id_sb = const.tile([P, P], BF16, tag="ident")

    def late_loads():
        nc.sync.dma_start(mask_sb[:], maskd)
        nc.sync.dma_start(id_sb[:], identd)
        nc.sync.dma_start(wo_sb[:], woT.rearrange("(c p) j -> p c j", p=P))


    qt = [qkp.tile([P, S], BF16, tag=f"qt{i}", name=f"qt{i}") for i in range(2)]
    kt = [qkp.tile([P, S], BF16, tag=f"kt{i}", name=f"kt{i}") for i in range(2)]
    vt = [vtp.tile([P, S], BF16, tag="vt", name=f"vt{i}") for i in range(2)]
    # ones-augmented V: fp8 copy (padded to stride 80 -- DoubleRow needs
    # the chunk stride to be a multiple of 16 bytes) for the bulk P@V,
    # plus a bf16 copy of key chunks 0-3 for the early query blocks
    # whose peaked softmax can't average out fp8 noise.
    vaug = [vaugp.tile([P, 16, 80], FP8, tag=f"vaug{h}", name=f"vaug{h}") for h in range(4)]
    vaug_hi = [vaugp.tile([P, 4, 65], BF16, tag=f"vaughi{h}", name=f"vaughi{h}") for h in range(4)]
    for h in range(4):
        nc.gpsimd.memset(vaug[h][:, :, 64:65], 1.0)
        nc.gpsimd.memset(vaug_hi[h][:, :, 64:65], 1.0)
    attn = [attnp.tile([P, S], BF16, tag=f"attn{i}", name=f"attn{i}") for i in range(2)]

    def qkv_group(pair, ci, sb):
        """One (chunk, s-block) accumulation of the QKV projection."""
        pc = [pair, 2 + pair, 4 + pair][ci]
        if pc < 4:
            w_ch = wqk_c[pc][:, :, :]
        else:
            w_ch = wv_c[pc - 4][:, :, :]
        dst = [qt[pair], kt[pair], vt[pair]][ci]
        ps = psml.tile([P, 512], F32, tag="psml", name="psml")
        for o in range(8):
            nc.tensor.matmul(
                ps[:],
                w_ch[:, o, :],
                x_t[sb][:, o, :],
                start=(o == 0),
                stop=(o == 7),
            )
        nc.vector.tensor_scalar(
            out=dst[:, 512 * sb : 512 * (sb + 1)],
            in0=ps[:],
            scalar1=b_sb[:, pc : pc + 1],
            scalar2=None,
            op0=add,
        )

    def v_finish(pair):
        """V_T -> V[k,d] via DRAM-bounce xbar transpose (contiguous dst),
        then engine-copy into the ones-augmented tiles."""
        nc.sync.dma_start(vdram[pair], vt[pair][:])
        for sub in range(2):
            h = 2 * pair + sub
            vkd = vtp.tile([P, 16, 64], BF16, tag="vkd", name=f"vkd{h}")
            nc.sync.dma_start_transpose(
                vkd[:], vdram[pair][64 * sub : 64 * sub + 64, :]
            )
            nc.vector.tensor_copy(vaug[h][:, :, 0:64], vkd[:])
            nc.vector.tensor_copy(vaug_hi[h][:, :, 0:64], vkd[:, 0:4, :])

    def filler_items(pair, T):
        for ci in range(3):
            for sb in range(4):
                yield ("qkv", pair, ci, sb)
        yield ("vfin", pair)
        for kc in range(4):
            yield ("esc", pair, kc, T)

    def emit_filler(it):
        if it is None:
            return
        if it[0] == "qkv":
            qkv_group(it[1], it[2], it[3])
        elif it[0] == "op":
            outproj_group(it[1], it[2], it[3])
        elif it[0] == "opdma":
            outproj_dma(it[1], it[2])
        elif it[0] == "esc":
            score_chunk(it[1], it[2], it[3])
        else:
            v_finish(it[1])

    def make_pt_tiles(pair):
        """P_T tiles for one pair: fp8 fixed-stride storage for key
        chunks 4-15 (pt), bf16 for chunks 0-3 x cols [0,1024) (pt_hi,
        full precision for the early query blocks whose peaked softmax
        can't average out fp8 noise), and per-pair fp8 scratch for
        chunks 0-3 x cols [1024,2048) (ptx) so the next pair's lead
        chunks can be computed early without aliasing this pair's."""
        pt = [
            pp.tile([P, 12, S], FP8, tag=f"pt{s}", name=f"pt{s}")
            for s in range(2)
        ]
        pt_hi = [
            pp.tile([P, 4, 1024], BF16, tag=f"pthi{s}", name=f"pthi{s}")
            for s in range(2)
        ]
        ptx = [
            pp.tile([P, 4, 1024], FP8, tag=f"ptx{pair}{s}", name=f"ptx{s}")
            for s in range(2)
        ]
        return pt, pt_hi, ptx

    def score_chunk(pair, kc, T):
        """Scores + exp + strip-zeroing for one 128-key chunk."""
        pt, pt_hi, ptx = T
        j = kc // 4
        d = kc - 4 * j
        g0 = (128 * kc) // 1024
        st = {}
        for sub in range(2):
            for g in range(g0, 2):
                st[sub, g] = scps.tile([P, 1024], F32, tag="st", name="st")
        # causal-mask init of the diagonal 128x128 block
        # (start=True clears the whole containing bank)
        lc = 128 * kc - 1024 * g0
        for sub in range(2):
            nc.tensor.matmul(
                st[sub, g0][:, lc : lc + 128],
                id_sb,
                mask_sb,
                start=True,
                stop=False,
            )
        # scores S_T[k, q], heads interleaved (PE row packing).
        # Diagonal block split at the 128-col boundary: first 128
        # cols accumulate onto the mask init, the rest of the bank
        # overwrites (has_written clear).
        for jb in range(j, 4):
            segs = []
            if jb == j:
                segs.append((128 * kc, 128, False, d == 3))
                if d < 3:
                    segs.append(
                        (128 * kc + 128, 512 * (j + 1) - 128 * kc - 128,
                         False, True)
                    )
            else:
                segs.append((512 * jb, 512, True, True))
            for n0, ln, sflag, eflag in segs:
                g = n0 // 1024
                l0 = n0 - 1024 * g
                for sub in range(2):
                    o0 = 64 * sub
                    nc.tensor.matmul(
                        st[sub, g][:, l0 : l0 + ln],
                        kt[pair][o0 : o0 + 64, 128 * kc : 128 * kc + 128],
                        qt[pair][o0 : o0 + 64, n0 : n0 + ln],
                        start=sflag,
                        stop=eflag,
                    )
        # exp (scaled) PSUM -> P_T
        for sub in range(2):
            for g in range(g0, 2):
                l0 = max(0, 128 * kc - 1024 * g)
                q0 = 1024 * g + l0
                gl = 1024 - l0
                if kc < 4:
                    dst = (
                        pt_hi[sub][:, kc, q0 : q0 + gl]
                        if g == 0
                        else ptx[sub][:, kc, q0 - 1024 : q0 - 1024 + gl]
                    )
                else:
                    dst = pt[sub][:, kc - 4, q0 : q0 + gl]
                nc.scalar.activation(
                    dst,
                    st[sub, g][:, l0 : l0 + gl],
                    Exp,
                    scale=SCALE,
                )
        # zero the below-diagonal strip [512j, 128kc)
        if d > 0:
            for sub in range(2):
                if kc < 4:
                    nc.gpsimd.memset(pt_hi[sub][:, kc, 0 : 128 * kc], 0.0)
                else:
                    nc.gpsimd.memset(
                        pt[sub][:, kc - 4, 512 * j : 128 * kc], 0.0
                    )

    def attention(pair, filler, T, per_j=None, lead=0, fills=None):
        """Attention for one head pair; pulls filler work between score
        chunks to keep the PE warm while ScalarE works through the exp
        backlog. lead: number of chunks already emitted earlier (as
        filler inside the previous pair's attention / right after it)."""
        for j in range(4):
            for kc in range(4 * j, 4 * j + 4):
                if kc >= lead:
                    score_chunk(pair, kc, T)
                for _ in range(fills[kc] if fills else 1):
                    emit_filler(next(filler, None))
            pv_block(pair, j, T)
            if per_j is not None:
                per_j(j)

    def pv_block(pair, j, T):
        pt, pt_hi, ptx = T
        if True:
            # P@V for query block j (plus denominator row 64). Key
            # chunks 0-3 of the first two query blocks run in bf16 from
            # pt_hi; the rest contract fp8 chunk-pairs via DoubleRow.
            for sub in range(2):
                h = 2 * pair + sub
                pv = psml.tile([P, 512], F32, tag="psml", name="pv")
                if j < 2:
                    for kc in range(4):
                        nc.tensor.matmul(
                            pv[0:65, :],
                            vaug_hi[h][:, kc, :],
                            pt_hi[sub][:, kc, 512 * j : 512 * (j + 1)],
                            start=(kc == 0),
                            stop=(j == 0 and kc == 3),
                        )
                for kp in range(2 * j + 2):
                    if kp < 2 and j < 2:
                        continue
                    if kp < 2:
                        rhs = ptx[sub][:, 2 * kp : 2 * kp + 2,
                                       512 * (j - 2) : 512 * (j - 1)]
                    else:
                        rhs = pt[sub][:, 2 * kp - 4 : 2 * kp - 2,
                                      512 * j : 512 * (j + 1)]
                    nc.tensor.matmul(
                        pv[0:65, :],
                        vaug[h][:, 2 * kp : 2 * kp + 2, 0:65],
                        rhs,
                        start=(kp == 0 and j >= 2),
                        stop=(kp == 2 * j + 1),
                        perf_mode=DR,
                    )
                sums = smallp.tile([1, 512], F32, tag="sums", name="sums")
                nc.vector.tensor_copy(sums[:], pv[64:65, :])
                rec = smallp.tile([1, 512], F32, tag="rec", name="rec")
                nc.vector.reciprocal_approx_fast(rec[:], sums[:])
                rbc = smallp.tile([64, 512], F32, tag="rbc", name="rbc")
                nc.gpsimd.partition_broadcast(rbc[:], rec[:])
                nc.vector.tensor_tensor(
                    out=attn[pair][64 * sub : 64 * sub + 64, 512 * j : 512 * (j + 1)],
                    in0=pv[0:64, :],
                    in1=rbc[:],
                    op=mult,
                )

    o_r = outT.rearrange("(o p) s -> p o s", p=P)

    def outproj_group(jc, sb, ev):
        """One (j-chunk, s-block) of the partial output projection."""
        ps = psml.tile([P, 512], F32, tag="psml", name="ops")
        for pc2 in range(2):
            nc.tensor.matmul(
                ps[:],
                wo_sb[:, pc2, 128 * jc : 128 * (jc + 1)],
                attn[pc2][:, 512 * sb : 512 * (sb + 1)],
                start=(pc2 == 0),
                stop=(pc2 == 1),
            )
        nc.vector.tensor_copy(ev[:, jc, :], ps[:])

    def outproj_dma(sb, ev):
        nc.sync.dma_start(o_r[:, :, 512 * sb : 512 * (sb + 1)], ev[:])

    # pair 0: QKV streamed per s-block right behind its x DMA, attention
    # with pair-1 QKV + pair-1's lead score chunks as PE filler.
    # pair 1: attention with the out-projection as PE filler -- each
    # finished s-block's 8 projection groups are queued and drained
    # through the next blocks' score slots, so the in-order PE never
    # stalls behind a normalization chain.
    # DMA issue order = HBM service order (single sync queue, FIFO).
    # First QKV group needs b (already queued), the pair-0 q-weight
    # chunk and x0 -- everything else streams behind.
    w_load(0)
    x_load(0)
    w_load(2)
    w_load(4)
    for sb in range(1, 4):
        x_load(sb)
    for pc in (1, 3, 5):
        w_load(pc)
    # Warm-up matmuls on a zeroed scratch tile keep the PE's HAM
    # activity window busy during the input-DMA wait: the ~3.4us
    # half-clock ramp is paid on throwaway work, and the real QKV
    # starts at 2.4GHz the moment x0 lands. Output goes to a psml
    # bank that the QKV groups immediately recycle.
    warm = xp.tile([P, 512], BF16, tag="warm")
    nc.gpsimd.memset(warm[:], 0.0)
    wps = psml.tile([P, 512], F32, tag="psml", name="warmps")
    for _ in range(NWARM):
        nc.tensor.matmul(wps[:], warm[:, 0:128], warm[:], start=True, stop=True)
    for sb in range(4):
        for ci in range(3):
            qkv_group(0, ci, sb)
    late_loads()
    v_finish(0)
    T0 = make_pt_tiles(0)
    T1 = make_pt_tiles(1)
    f1 = filler_items(1, T1)
    attention(0, f1, T0, fills=[1] * 8 + [2] * 8)
    for it in f1:
        emit_filler(it)
    # pair 1's chunks 4-7 right behind pair 0's last P@V: their exps
    # overwrite pair-0 P_T regions, so they can't go earlier, but
    # emitting them here keeps PE and ScalarE primed through the
    # pair transition.
    for kc in range(4, 8):
        score_chunk(1, kc, T1)

    import collections as _collections
    import itertools as _itertools

    oq = _collections.deque()

    def push_outproj(j):
        ev = oevacp.tile([P, 8, 512], F16, tag="evb", name=f"evb{j}", bufs=2)
        oq.extend(("op", jc, j, ev) for jc in range(8))
        oq.append(("opdma", j, ev))

    attention(
        1,
        ((oq.popleft() if oq else None) for _ in _itertools.count()),
        T1,
        per_j=push_outproj,
        lead=8,
        fills=[2] * 16,
    )
    while oq:
        emit_filler(oq.popleft())


_NC_CACHE = {}


def build_nc():
    if "nc" in _NC_CACHE:
        return _NC_CACHE["nc"]
    nc = bacc.Bacc(
        "TRN2",
        target_bir_lowering=False,
        debug=False,
        num_devices=8,
    )
    with tile.TileContext(nc) as tc:
        with ExitStack() as ctx:
            _emit(nc, tc, ctx)
    nc.compile()
    _NC_CACHE["nc"] = nc
    return nc


def make_in_maps(hidden_states, w_in, b_in, w_out):
    hidden_states = np.asarray(hidden_states, dtype=np.float32)
    w_in = np.asarray(w_in, dtype=np.float32)
    b_in = np.asarray(b_in, dtype=np.float32)
    w_out = np.asarray(w_out, dtype=np.float32)

    xT = [np.ascontiguousarray(hidden_states[b].T).astype(BF16NP) for b in range(B)]
    mask = np.where(
        np.arange(P)[:, None] <= np.arange(P)[None, :], 0.0, NEG
    ).astype(BF16NP)
    ident = np.eye(P, dtype=BF16NP)

    in_maps = []
    for c in range(8):
        b, hg = divmod(c, 4)
        q0 = 256 * hg
        wq = w_in[q0 : q0 + 256]
        wk = w_in[H + q0 : H + q0 + 256]
        wv = w_in[2 * H + q0 : 2 * H + q0 + 256]
        in_maps.append(
            {
                "xT": xT[b],
                "wqkT": np.ascontiguousarray(
                    np.concatenate([wq, wk], 0).T
                ).astype(BF16NP),
                "wvT": np.ascontiguousarray(wv.T).astype(BF16NP),
                "bqkv": np.ascontiguousarray(
                    np.concatenate(
                        [b_in[q0 : q0 + 256], b_in[H + q0 : H + q0 + 256],
                         b_in[2 * H + q0 : 2 * H + q0 + 256]]
                    )
                ),
                "woT": np.ascontiguousarray(w_out[:, q0 : q0 + 256].T).astype(BF16NP),
                "maskd": mask,
                "identd": ident,
            }
        )
    return in_maps


def _ensure_ntff_hook():
    """Provide antenv.axon_hooks (NTFF profiling hook) if the container's
    antenv stub lacks it, by driving the axon .so C ABI directly. Also
    neuter the S3 artifact upload (zero-egress container)."""
    import contextlib
    import ctypes
    import types

    import concourse.bass_utils as bu

    bu.upload_artifacts = lambda tmpdir: str(tmpdir)
    try:
        from antenv.axon_hooks import get_axon_ntff_profile_hook  # noqa: F401

        return
    except ImportError:
        pass
    import antenv

    so_path = "/opt/axon/libaxon_pjrt.so"
    hook = None
    try:
        lib = ctypes.CDLL(so_path)
        if hasattr(lib, "axon_start_nrt_profile"):
            lib.axon_start_nrt_profile.argtypes = [
                ctypes.POINTER(ctypes.c_int64),
                ctypes.c_size_t,
            ]
            lib.axon_start_nrt_profile.restype = ctypes.c_int64
            lib.axon_stop_nrt_profile.argtypes = [ctypes.c_char_p]
            lib.axon_stop_nrt_profile.restype = ctypes.c_int64

            @contextlib.contextmanager
            def _hook(output_dir, device_ids):
                import jax

                jax.devices()
                if device_ids:
                    ids = (ctypes.c_int64 * len(device_ids))(*device_ids)
                    rc = lib.axon_start_nrt_profile(ids, len(device_ids))
                else:
                    rc = lib.axon_start_nrt_profile(None, 0)
                if rc != 0:
                    raise RuntimeError(f"axon_start_nrt_profile rc={rc}")
                try:
                    yield
                finally:
                    n = lib.axon_stop_nrt_profile(str(output_dir).encode())
                    print(f"ntff profile: {n} file(s) -> {output_dir}")

            hook = _hook
    except OSError:
        hook = None

    mod = types.ModuleType("antenv.axon_hooks")
    mod.get_axon_ntff_profile_hook = lambda: hook
    mod.set_axon_ntff_profile_hook = lambda h: None
    sys.modules["antenv.axon_hooks"] = mod
    antenv.axon_hooks = mod


def run_device(hidden_states, w_in, b_in, w_out, b_out, trace=False):
    """Returns (full output, BassKernelResults)."""
    if trace:
        _ensure_ntff_hook()
    nc = build_nc()
    in_maps = make_in_maps(hidden_states, w_in, b_in, w_out)
    res = run_bass_kernel_spmd(
        nc, in_maps, core_ids=list(range(8)), trace=trace
    )
    out = np.zeros((B, S, H), dtype=np.float32)
    for c in range(8):
        out[c // 4] += res.results[c]["outT"].T
    out += np.asarray(b_out, dtype=np.float32)[None, None, :]
    return out, res


def kernel(hidden_states, w_in, b_in, w_out, b_out):
    out, _ = run_device(hidden_states, w_in, b_in, w_out, b_out, trace=False)
    return out



# revision 12
# speedup vs baseline: 1.0765x; 1.0226x over previous
"""Bass/Trainium2 kernel for nn_MultiHeadAttention_85615878078828.

Full (unsharded) inputs in, full output out. Sharding: 8 cores =
2 batches x 4 head-groups (tensor-parallel on heads + data-parallel on
batch). Each core runs QKV projection + causal attention for its 4
heads + a partial output projection; the host sums the 4 partial
projections per batch element (the "all-reduce" of the TP out-proj,
done during unshard) and adds b_out.

Everything on-device is computed in transposed layout ([dim, seq]) so
no on-device transposes of activations are needed:
  - QKV_T  = W-stationary matmuls against x^T (host pre-transposes x)
  - scores = S_T[k, q] directly (2 heads packed on the PE array via
    row tiling, contract dim = 64 each)
  - causal mask folded into PSUM with an identity-matmul init
  - exp on ScalarE with fused 1/sqrt(D) scale, P_T stored bf16
  - P@V with ones-augmented V (M=65) -> softmax denominators fall out
    of the same accumulation (row 64)
  - normalization: reciprocal + gpsimd partition-broadcast + multiply
  - out-proj in transposed layout; host transposes back.
"""

import os
import sys
from contextlib import ExitStack

import numpy as np

sys.path.insert(0, "/opt/trn_rl_repo")

import ml_dtypes

BF16NP = ml_dtypes.bfloat16
F8NP = ml_dtypes.float8_e4m3

import concourse.bass as bass
import concourse.tile as tile
from concourse import bacc, mybir
from concourse.bass_utils import run_bass_kernel_spmd

# Problem shapes (hardcoded per contract)
B, S, H, N = 2, 2048, 1024, 16
D = H // N  # 64
P = 128
SCALE = float(D) ** -0.5  # 0.125
NEG = -1.0e30

F32 = mybir.dt.float32
F32R = mybir.dt.float32r
BF16 = mybir.dt.bfloat16
F16 = mybir.dt.float16
FP8 = mybir.dt.float8e4
DR = mybir.MatmulPerfMode.DoubleRow
DEBUG = False
NWARM = 17


def _emit(nc, tc, ctx):
    add = mybir.AluOpType.add
    mult = mybir.AluOpType.mult
    Exp = mybir.ActivationFunctionType.Exp

    # ---- DRAM I/O ----
    xT = nc.dram_tensor("xT", [H, S], BF16, kind="ExternalInput").ap()
    wqkT = nc.dram_tensor("wqkT", [H, 512], BF16, kind="ExternalInput").ap()
    wvT = nc.dram_tensor("wvT", [H, 256], BF16, kind="ExternalInput").ap()
    bqkv = nc.dram_tensor("bqkv", [768], F32, kind="ExternalInput").ap()
    woT = nc.dram_tensor("woT", [256, H], BF16, kind="ExternalInput").ap()
    maskd = nc.dram_tensor("maskd", [P, P], BF16, kind="ExternalInput").ap()
    identd = nc.dram_tensor("identd", [P, P], BF16, kind="ExternalInput").ap()
    outT = nc.dram_tensor("outT", [H, S], F16, kind="ExternalOutput").ap()
    vdram = [nc.dram_tensor(f"vdram{i}", [P, S], BF16).ap() for i in range(2)]

    # ---- pools (single scope; SBUF is sized to fit everything) ----
    const = ctx.enter_context(tc.tile_pool(name="const", bufs=1))
    qkp = ctx.enter_context(tc.tile_pool(name="qk", bufs=1))
    vaugp = ctx.enter_context(tc.tile_pool(name="vaug", bufs=1))
    attnp = ctx.enter_context(tc.tile_pool(name="attn", bufs=1))
    smallp = ctx.enter_context(tc.tile_pool(name="small", bufs=3))
    oevacp = ctx.enter_context(tc.tile_pool(name="oevac", bufs=4))
    xp = ctx.enter_context(tc.tile_pool(name="xp", bufs=1))
    vtp = ctx.enter_context(tc.tile_pool(name="vtp", bufs=2))
    pp = ctx.enter_context(tc.tile_pool(name="pp", bufs=1))
    # PSUM: scores 2x(2 banks) + shared small-psum 4x(1 bank) = 8 banks
    scps = ctx.enter_context(tc.tile_pool(name="scps", bufs=2, space="PSUM"))
    psml = ctx.enter_context(tc.tile_pool(name="psml", bufs=4, space="PSUM"))

    # DMA order is tuned so the first QKV group's inputs (b, q-weight
    # chunk for pair 0, x s-block 0) land first; everything else
    # streams behind at full HBM rate. Weight chunks live in separate
    # tiles because Tile tracks DMA->matmul deps per tile.
    b_sb = xp.tile([P, 6], F32, tag="b")
    nc.sync.dma_start(b_sb[:], bqkv.rearrange("(c p) -> p c", p=P))
    _wqk_r = wqkT.rearrange("(o p) c -> p o c", p=P)
    _wv_r = wvT.rearrange("(o p) c -> p o c", p=P)
    wqk_c = [
        xp.tile([P, 8, 128], BF16, tag=f"wqk{pc}", name=f"wqk{pc}")
        for pc in range(4)
    ]
    wv_c = [
        xp.tile([P, 8, 128], BF16, tag=f"wv{pc}", name=f"wv{pc}")
        for pc in range(2)
    ]

    def w_load(pc):
        if pc < 4:
            nc.sync.dma_start(
                wqk_c[pc][:], _wqk_r[:, :, 128 * pc : 128 * (pc + 1)]
            )
        else:
            nc.sync.dma_start(
                wv_c[pc - 4][:], _wv_r[:, :, 128 * (pc - 4) : 128 * (pc - 3)]
            )

    # one tile PER s-block: DMA-write -> matmul-read dependencies are
    # tracked per tile, so a single x tile would gate every QKV group
    # on the LAST x DMA (measured: first matmul sat idle 7.7us for it)
    x_t = [
        xp.tile([P, 8, 512], BF16, tag=f"x{sb}", name=f"x{sb}")
        for sb in range(4)
    ]
    x_r = xT.rearrange("(o p) s -> p o s", p=P)

    def x_load(sb):
        nc.sync.dma_start(
            x_t[sb][:], x_r[:, :, 512 * sb : 512 * (sb + 1)]
        )

    wo_sb = const.tile([P, 2, H], BF16, tag="wo")
    mask_sb = const.tile([P, P], BF16, tag="mask")
    id_sb = const.tile([P, P], BF16, tag="ident")

    def late_loads():
        nc.sync.dma_start(mask_sb[:], maskd)
        nc.sync.dma_start(id_sb[:], identd)
        nc.sync.dma_start(wo_sb[:], woT.rearrange("(c p) j -> p c j", p=P))


    qt = [qkp.tile([P, S], BF16, tag=f"qt{i}", name=f"qt{i}") for i in range(2)]
    kt = [qkp.tile([P, S], BF16, tag=f"kt{i}", name=f"kt{i}") for i in range(2)]
    vt = [vtp.tile([P, S], BF16, tag="vt", name=f"vt{i}") for i in range(2)]
    # ones-augmented V: fp8 copy (padded to stride 80 -- DoubleRow needs
    # the chunk stride to be a multiple of 16 bytes) for the bulk P@V,
    # plus a bf16 copy of key chunks 0-3 for the early query blocks
    # whose peaked softmax can't average out fp8 noise.
    vaug = [vaugp.tile([P, 16, 80], FP8, tag=f"vaug{h}", name=f"vaug{h}") for h in range(4)]
    vaug_hi = [vaugp.tile([P, 4, 65], BF16, tag=f"vaughi{h}", name=f"vaughi{h}") for h in range(4)]
    for h in range(4):
        nc.gpsimd.memset(vaug[h][:, :, 64:65], 1.0)
        nc.gpsimd.memset(vaug_hi[h][:, :, 64:65], 1.0)
    attn = [attnp.tile([P, S], BF16, tag=f"attn{i}", name=f"attn{i}") for i in range(2)]

    def qkv_group(pair, ci, sb):
        """One (chunk, s-block) accumulation of the QKV projection."""
        pc = [pair, 2 + pair, 4 + pair][ci]
        if pc < 4:
            w_ch = wqk_c[pc][:, :, :]
        else:
            w_ch = wv_c[pc - 4][:, :, :]
        dst = [qt[pair], kt[pair], vt[pair]][ci]
        ps = psml.tile([P, 512], F32, tag="psml", name="psml")
        for o in range(8):
            nc.tensor.matmul(
                ps[:],
                w_ch[:, o, :],
                x_t[sb][:, o, :],
                start=(o == 0),
                stop=(o == 7),
            )
        nc.vector.tensor_scalar(
            out=dst[:, 512 * sb : 512 * (sb + 1)],
            in0=ps[:],
            scalar1=b_sb[:, pc : pc + 1],
            scalar2=None,
            op0=add,
        )

    def v_finish(pair):
        """V_T -> V[k,d] via DRAM-bounce xbar transpose (contiguous dst),
        then engine-copy into the ones-augmented tiles."""
        nc.sync.dma_start(vdram[pair], vt[pair][:])
        for sub in range(2):
            h = 2 * pair + sub
            vkd = vtp.tile([P, 16, 64], BF16, tag="vkd", name=f"vkd{h}")
            nc.sync.dma_start_transpose(
                vkd[:], vdram[pair][64 * sub : 64 * sub + 64, :]
            )
            nc.vector.tensor_copy(vaug[h][:, :, 0:64], vkd[:])
            nc.vector.tensor_copy(vaug_hi[h][:, :, 0:64], vkd[:, 0:4, :])

    def filler_items(pair, T):
        for ci in range(3):
            for sb in range(4):
                yield ("qkv", pair, ci, sb)
        yield ("vfin", pair)
        for kc in range(4):
            yield ("esc", pair, kc, T)

    def emit_filler(it):
        if it is None:
            return
        if it[0] == "qkv":
            qkv_group(it[1], it[2], it[3])
        elif it[0] == "op":
            outproj_group(it[1], it[2], it[3])
        elif it[0] == "opdma":
            outproj_dma(it[1], it[2])
        elif it[0] == "opdma2":
            outproj_dma2(it[1], it[2], it[3])
        elif it[0] == "esc":
            score_chunk(it[1], it[2], it[3])
        else:
            v_finish(it[1])

    def make_pt_tiles(pair):
        """P_T tiles for one pair: fp8 fixed-stride storage for key
        chunks 4-15 (pt), bf16 for chunks 0-3 x cols [0,1024) (pt_hi,
        full precision for the early query blocks whose peaked softmax
        can't average out fp8 noise), and per-pair fp8 scratch for
        chunks 0-3 x cols [1024,2048) (ptx) so the next pair's lead
        chunks can be computed early without aliasing this pair's."""
        pt = [
            pp.tile([P, 12, S], FP8, tag=f"pt{s}", name=f"pt{s}")
            for s in range(2)
        ]
        pt_hi = [
            pp.tile([P, 4, 1024], BF16, tag=f"pthi{s}", name=f"pthi{s}")
            for s in range(2)
        ]
        ptx = [
            pp.tile([P, 4, 1024], FP8, tag=f"ptx{pair}{s}", name=f"ptx{s}")
            for s in range(2)
        ]
        return pt, pt_hi, ptx

    def score_chunk(pair, kc, T):
        """Scores + exp + strip-zeroing for one 128-key chunk."""
        pt, pt_hi, ptx = T
        j = kc // 4
        d = kc - 4 * j
        g0 = (128 * kc) // 1024
        st = {}
        for sub in range(2):
            for g in range(g0, 2):
                st[sub, g] = scps.tile([P, 1024], F32, tag="st", name="st")
        # causal-mask init of the diagonal 128x128 block
        # (start=True clears the whole containing bank)
        lc = 128 * kc - 1024 * g0
        for sub in range(2):
            nc.tensor.matmul(
                st[sub, g0][:, lc : lc + 128],
                id_sb,
                mask_sb,
                start=True,
                stop=False,
            )
        # scores S_T[k, q], heads interleaved (PE row packing).
        # Diagonal block split at the 128-col boundary: first 128
        # cols accumulate onto the mask init, the rest of the bank
        # overwrites (has_written clear).
        for jb in range(j, 4):
            segs = []
            if jb == j:
                segs.append((128 * kc, 128, False, d == 3))
                if d < 3:
                    segs.append(
                        (128 * kc + 128, 512 * (j + 1) - 128 * kc - 128,
                         False, True)
                    )
            else:
                segs.append((512 * jb, 512, True, True))
            for n0, ln, sflag, eflag in segs:
                g = n0 // 1024
                l0 = n0 - 1024 * g
                for sub in range(2):
                    o0 = 64 * sub
                    nc.tensor.matmul(
                        st[sub, g][:, l0 : l0 + ln],
                        kt[pair][o0 : o0 + 64, 128 * kc : 128 * kc + 128],
                        qt[pair][o0 : o0 + 64, n0 : n0 + ln],
                        start=sflag,
                        stop=eflag,
                    )
        # exp (scaled) PSUM -> P_T
        for sub in range(2):
            for g in range(g0, 2):
                l0 = max(0, 128 * kc - 1024 * g)
                q0 = 1024 * g + l0
                gl = 1024 - l0
                if kc < 4:
                    dst = (
                        pt_hi[sub][:, kc, q0 : q0 + gl]
                        if g == 0
                        else ptx[sub][:, kc, q0 - 1024 : q0 - 1024 + gl]
                    )
                else:
                    dst = pt[sub][:, kc - 4, q0 : q0 + gl]
                nc.scalar.activation(
                    dst,
                    st[sub, g][:, l0 : l0 + gl],
                    Exp,
                    scale=SCALE,
                )
        # zero the below-diagonal strip [512j, 128kc)
        if d > 0:
            for sub in range(2):
                if kc < 4:
                    nc.gpsimd.memset(pt_hi[sub][:, kc, 0 : 128 * kc], 0.0)
                else:
                    nc.gpsimd.memset(
                        pt[sub][:, kc - 4, 512 * j : 128 * kc], 0.0
                    )

    def attention(pair, filler, T, per_j=None, lead=0, fills=None):
        """Attention for one head pair; pulls filler work between score
        chunks to keep the PE warm while ScalarE works through the exp
        backlog. lead: number of chunks already emitted earlier (as
        filler inside the previous pair's attention / right after it)."""
        for j in range(4):
            for kc in range(4 * j, 4 * j + 4):
                if kc >= lead:
                    score_chunk(pair, kc, T)
                for _ in range(fills[kc] if fills else 1):
                    emit_filler(next(filler, None))
            pv_block(pair, j, T)
            if per_j is not None:
                per_j(j)

    def pv_block(pair, j, T):
        pt, pt_hi, ptx = T
        if True:
            # P@V for query block j (plus denominator row 64). Key
            # chunks 0-3 of the first two query blocks run in bf16 from
            # pt_hi; the rest contract fp8 chunk-pairs via DoubleRow.
            for sub in range(2):
                h = 2 * pair + sub
                pv = psml.tile([P, 512], F32, tag="psml", name="pv")
                if j < 2:
                    for kc in range(4):
                        nc.tensor.matmul(
                            pv[0:65, :],
                            vaug_hi[h][:, kc, :],
                            pt_hi[sub][:, kc, 512 * j : 512 * (j + 1)],
                            start=(kc == 0),
                            stop=(j == 0 and kc == 3),
                        )
                for kp in range(2 * j + 2):
                    if kp < 2 and j < 2:
                        continue
                    if kp < 2:
                        rhs = ptx[sub][:, 2 * kp : 2 * kp + 2,
                                       512 * (j - 2) : 512 * (j - 1)]
                    else:
                        rhs = pt[sub][:, 2 * kp - 4 : 2 * kp - 2,
                                      512 * j : 512 * (j + 1)]
                    nc.tensor.matmul(
                        pv[0:65, :],
                        vaug[h][:, 2 * kp : 2 * kp + 2, 0:65],
                        rhs,
                        start=(kp == 0 and j >= 2),
                        stop=(kp == 2 * j + 1),
                        perf_mode=DR,
                    )
                # copy to SBUF first: reciprocal_approx_fast's bit-trick
                # seed needs IEEE fp32, not PSUM's raw accumulator bits
                sums = smallp.tile([1, 512], F32, tag="sums", name="sums")
                nc.vector.tensor_copy(sums[:], pv[64:65, :])
                rec = smallp.tile([1, 512], F32, tag="rec", name="rec")
                nc.vector.reciprocal_approx_fast(rec[:], sums[:])
                rbc = smallp.tile([64, 512], F32, tag="rbc", name="rbc")
                nc.gpsimd.partition_broadcast(rbc[:], rec[:])
                nc.vector.tensor_tensor(
                    out=attn[pair][64 * sub : 64 * sub + 64, 512 * j : 512 * (j + 1)],
                    in0=pv[0:64, :],
                    in1=rbc[:],
                    op=mult,
                )

    o_r = outT.rearrange("(o p) s -> p o s", p=P)

    def outproj_group(jc, sb, ev):
        """One (j-chunk, s-block) of the partial output projection."""
        ps = psml.tile([P, 512], F32, tag="psml", name="ops")
        for pc2 in range(2):
            nc.tensor.matmul(
                ps[:],
                wo_sb[:, pc2, 128 * jc : 128 * (jc + 1)],
                attn[pc2][:, 512 * sb : 512 * (sb + 1)],
                start=(pc2 == 0),
                stop=(pc2 == 1),
            )
        nc.vector.tensor_copy(ev[:, jc, :], ps[:])

    def outproj_dma(sb, ev):
        nc.sync.dma_start(o_r[:, :, 512 * sb : 512 * (sb + 1)], ev[:])

    def outproj_dma2(sb, ev, half):
        nc.sync.dma_start(
            o_r[:, 4 * half : 4 * half + 4, 512 * sb : 512 * (sb + 1)],
            ev[:, 4 * half : 4 * half + 4, :],
        )

    # pair 0: QKV streamed per s-block right behind its x DMA, attention
    # with pair-1 QKV + pair-1's lead score chunks as PE filler.
    # pair 1: attention with the out-projection as PE filler -- each
    # finished s-block's 8 projection groups are queued and drained
    # through the next blocks' score slots, so the in-order PE never
    # stalls behind a normalization chain.
    # DMA issue order = HBM service order (single sync queue, FIFO).
    # First QKV group needs b (already queued), the pair-0 q-weight
    # chunk and x0 -- everything else streams behind.
    w_load(0)
    x_load(0)
    w_load(2)
    w_load(4)
    for sb in range(1, 4):
        x_load(sb)
    for pc in (1, 3, 5):
        w_load(pc)
    # Warm-up matmuls on a zeroed scratch tile keep the PE's HAM
    # activity window busy during the input-DMA wait: the ~3.4us
    # half-clock ramp is paid on throwaway work, and the real QKV
    # starts at 2.4GHz the moment x0 lands. Output goes to a psml
    # bank that the QKV groups immediately recycle.
    warm = xp.tile([P, 512], BF16, tag="warm")
    nc.gpsimd.memset(warm[:], 0.0)
    wps = psml.tile([P, 512], F32, tag="psml", name="warmps")
    for _ in range(NWARM):
        nc.tensor.matmul(wps[:], warm[:, 0:128], warm[:], start=True, stop=True)
    for sb in range(4):
        for ci in range(3):
            qkv_group(0, ci, sb)
    late_loads()
    v_finish(0)
    T0 = make_pt_tiles(0)
    T1 = make_pt_tiles(1)
    f1 = filler_items(1, T1)
    attention(0, f1, T0, fills=[1] * 8 + [2] * 8)
    for it in f1:
        emit_filler(it)
    # pair 1's chunks 4-7 right behind pair 0's last P@V: their exps
    # overwrite pair-0 P_T regions, so they can't go earlier, but
    # emitting them here keeps PE and ScalarE primed through the
    # pair transition.
    for kc in range(4, 8):
        score_chunk(1, kc, T1)

    import collections as _collections

    # pair-1 attention, explicitly scheduled: each j's P@V +
    # normalization chain and the out-projection groups it unlocks are
    # hidden behind score chunks emitted one j ahead of their consumer
    # (chunks 8-11 for the future j=2 run during j=0, etc), so the PE
    # never drains while Vector/GpSimd work through a normalization.
    def op_items(j):
        ev = oevacp.tile([P, 8, 512], F16, tag="evb", name=f"evb{j}", bufs=2)
        items = [("op", jc, j, ev) for jc in range(8)]
        items.insert(4, ("opdma2", j, ev, 0))
        items.append(("opdma2", j, ev, 1))
        return _collections.deque(items)

    carry = _collections.deque()
    for jv in range(4):
        pv_block(1, jv, T1)
        for i, kc in enumerate({0: (8, 9, 10, 11), 1: (12, 13, 14, 15)}.get(jv, ())):
            score_chunk(1, kc, T1)
            if i >= 1:
                for _ in range(3):
                    if carry:
                        emit_filler(carry.popleft())
        while carry:
            emit_filler(carry.popleft())
        carry = op_items(jv)
    while carry:
        emit_filler(carry.popleft())


_NC_CACHE = {}


def build_nc():
    if "nc" in _NC_CACHE:
        return _NC_CACHE["nc"]
    nc = bacc.Bacc(
        "TRN2",
        target_bir_lowering=False,
        debug=False,
        num_devices=8,
    )
    with tile.TileContext(nc) as tc:
        with ExitStack() as ctx:
            _emit(nc, tc, ctx)
    nc.compile()
    _NC_CACHE["nc"] = nc
    return nc


def make_in_maps(hidden_states, w_in, b_in, w_out):
    hidden_states = np.asarray(hidden_states, dtype=np.float32)
    w_in = np.asarray(w_in, dtype=np.float32)
    b_in = np.asarray(b_in, dtype=np.float32)
    w_out = np.asarray(w_out, dtype=np.float32)

    xT = [np.ascontiguousarray(hidden_states[b].T).astype(BF16NP) for b in range(B)]
    mask = np.where(
        np.arange(P)[:, None] <= np.arange(P)[None, :], 0.0, NEG
    ).astype(BF16NP)
    ident = np.eye(P, dtype=BF16NP)

    in_maps = []
    for c in range(8):
        b, hg = divmod(c, 4)
        q0 = 256 * hg
        wq = w_in[q0 : q0 + 256]
        wk = w_in[H + q0 : H + q0 + 256]
        wv = w_in[2 * H + q0 : 2 * H + q0 + 256]
        in_maps.append(
            {
                "xT": xT[b],
                "wqkT": np.ascontiguousarray(
                    np.concatenate([wq, wk], 0).T
                ).astype(BF16NP),
                "wvT": np.ascontiguousarray(wv.T).astype(BF16NP),
                "bqkv": np.ascontiguousarray(
                    np.concatenate(
                        [b_in[q0 : q0 + 256], b_in[H + q0 : H + q0 + 256],
                         b_in[2 * H + q0 : 2 * H + q0 + 256]]
                    )
                ),
                "woT": np.ascontiguousarray(w_out[:, q0 : q0 + 256].T).astype(BF16NP),
                "maskd": mask,
                "identd": ident,
            }
        )
    return in_maps


def _ensure_ntff_hook():
    """Provide antenv.axon_hooks (NTFF profiling hook) if the container's
    antenv stub lacks it, by driving the axon .so C ABI directly. Also
    neuter the S3 artifact upload (zero-egress container)."""
    import contextlib
    import ctypes
    import types

    import concourse.bass_utils as bu

    bu.upload_artifacts = lambda tmpdir: str(tmpdir)
    try:
        from antenv.axon_hooks import get_axon_ntff_profile_hook  # noqa: F401

        return
    except ImportError:
        pass
    import antenv

    so_path = "/opt/axon/libaxon_pjrt.so"
    hook = None
    try:
        lib = ctypes.CDLL(so_path)
        if hasattr(lib, "axon_start_nrt_profile"):
            lib.axon_start_nrt_profile.argtypes = [
                ctypes.POINTER(ctypes.c_int64),
                ctypes.c_size_t,
            ]
            lib.axon_start_nrt_profile.restype = ctypes.c_int64
            lib.axon_stop_nrt_profile.argtypes = [ctypes.c_char_p]
            lib.axon_stop_nrt_profile.restype = ctypes.c_int64

            @contextlib.contextmanager
            def _hook(output_dir, device_ids):
                import jax

                jax.devices()
                if device_ids:
                    ids = (ctypes.c_int64 * len(device_ids))(*device_ids)
                    rc = lib.axon_start_nrt_profile(ids, len(device_ids))
                else:
                    rc = lib.axon_start_nrt_profile(None, 0)
                if rc != 0:
                    raise RuntimeError(f"axon_start_nrt_profile rc={rc}")
                try:
                    yield
                finally:
                    n = lib.axon_stop_nrt_profile(str(output_dir).encode())
                    print(f"ntff profile: {n} file(s) -> {output_dir}")

            hook = _hook
    except OSError:
        hook = None

    mod = types.ModuleType("antenv.axon_hooks")
    mod.get_axon_ntff_profile_hook = lambda: hook
    mod.set_axon_ntff_profile_hook = lambda h: None
    sys.modules["antenv.axon_hooks"] = mod
    antenv.axon_hooks = mod


def run_device(hidden_states, w_in, b_in, w_out, b_out, trace=False):
    """Returns (full output, BassKernelResults)."""
    if trace:
        _ensure_ntff_hook()
    nc = build_nc()
    in_maps = make_in_maps(hidden_states, w_in, b_in, w_out)
    res = run_bass_kernel_spmd(
        nc, in_maps, core_ids=list(range(8)), trace=trace
    )
    out = np.zeros((B, S, H), dtype=np.float32)
    for c in range(8):
        out[c // 4] += res.results[c]["outT"].T
    out += np.asarray(b_out, dtype=np.float32)[None, None, :]
    return out, res


def kernel(hidden_states, w_in, b_in, w_out, b_out):
    out, _ = run_device(hidden_states, w_in, b_in, w_out, b_out, trace=False)
    return out

